# revision 15
# baseline (speedup 1.0000x reference)
"""Trainium2 Bass kernel: scatter-add of table rows into a voxel grid.

Computes out[cell] += table[row] for ~1M (cell, row) events, out shape
[B*W*H*L, D] = [131072, 256] fp32.

Pairing strategy: the bottleneck is SWDGE descriptor generation on the
Pool engine (~2.6ns/descriptor, one descriptor per gathered event row).
To halve the descriptor count, events within a tile are sorted by table
row and paired on the fixed (even, odd) grid whenever the row gap of
the pair is <= 7. A host-built augmented table TDALL[d*4096 + a] =
[table[a], table[a+d]] (8 deltas x 4096 rows x 1KB = 32MB HBM) lets one
1KB descriptor fetch both rows of a pair. Unpaired events use delta=0
with the second half masked off via a -1 one-hot lane.

Device per pair-chunk (128 slots = up to 256 events): one dma_gather of
128 x 1KB, two fp8 one-hot builds (cells of first/second events), two
matmuls accumulating into the tile's PSUM bank. Output is downcast to
bf16, written partition-major, reassembled + upcast on host.
"""

import numpy as np
import ml_dtypes

B, W, H, L, D = 4, 32, 32, 32, 256
NCELLS = B * W * H * L          # 131072
TROWS = 4096
NCORES = 8
TPC = NCELLS // 128 // NCORES   # tile positions per core: 128
NDELTA = 8                      # pair row-gap range [0, 7]
GIDX = 1024                     # pair-slots per dma_gather call
GCH = GIDX // 128               # pair-chunks per gather call: 8
NSEG = 8                        # rows_w load segments (early gather start)
OHB = 8                         # one-hot builds batched per DVE op
OB = 8                          # output tiles batched per DMA

_compiled = {}


def _build(S):
    import concourse.tile as tile
    from concourse import bacc, mybir

    f32, bf16, i16 = mybir.dt.float32, mybir.dt.bfloat16, mybir.dt.int16
    f8 = mybir.dt.float8e4
    nch = int(sum(S))                    # pair-chunks per core
    assert nch % GCH == 0
    ncalls = nch // GCH
    cps = -(-ncalls // NSEG)             # gather calls per rows_w segment

    nc = bacc.Bacc("TRN2", target_bir_lowering=False, debug=False,
                   num_devices=NCORES, num_swdge_queues=4)
    tdall = nc.dram_tensor("tdall", [NDELTA * TROWS, 2 * D], bf16,
                           kind="ExternalInput")
    rows_w = nc.dram_tensor("rows_w", [128, ncalls * (GIDX // 16)], i16,
                            kind="ExternalInput")
    lrel = nc.dram_tensor("lrel", [128, 2, nch], bf16, kind="ExternalInput")
    out = nc.dram_tensor("out", [128, TPC, D], bf16, kind="ExternalOutput")

    with tile.TileContext(nc) as tc:
        with tc.tile_pool(name="const", bufs=1) as constp, \
             tc.tile_pool(name="rows", bufs=8) as rowsp, \
             tc.tile_pool(name="lrelp", bufs=8) as lrelp, \
             tc.tile_pool(name="gbuf", bufs=14) as gpool, \
             tc.tile_pool(name="oh", bufs=10) as ohpool, \
             tc.tile_pool(name="psum", bufs=8, space="PSUM") as pspool, \
             tc.tile_pool(name="stage", bufs=6) as stpool:
            # uneven segmentation: tiny first segments so the first gather
            # and one-hot build start within a few us; the rest in a few
            # large (DMA-efficient) loads
            CW = GIDX // 16
            q = max(1, (ncalls - 4) // 4)
            cuts = [0, 1, 2, 4, 4 + q, 4 + 2 * q, 4 + 3 * q, ncalls]
            cuts = sorted(set(min(x, ncalls) for x in cuts))
            rows_sb = []        # (start_call, tile) per segment
            for a, b in zip(cuts[:-1], cuts[1:]):
                t = rowsp.tile([128, (b - a) * CW], i16)
                nc.sync.dma_start(t[:], rows_w[:, a * CW:b * CW])
                rows_sb.append((a, b, t))

            def rows_slice(ci):
                for a, b, t in rows_sb:
                    if a <= ci < b:
                        return t[:, (ci - a) * CW:(ci - a + 1) * CW]
                raise AssertionError(ci)

            ql = max(OHB, ((nch - 2 * OHB) // (3 * OHB)) * OHB)
            lcuts = [0, OHB, 2 * OHB, 2 * OHB + ql, 2 * OHB + 2 * ql, nch]
            lcuts = sorted(set(min(x, nch) for x in lcuts))
            lrel_sb = []
            for a, b in zip(lcuts[:-1], lcuts[1:]):
                t = lrelp.tile([128, 2, b - a], bf16)
                nc.sync.dma_start(t[:], lrel[:, :, a:b])
                lrel_sb.append((a, b, t))

            def lrel_slice(c, nb):
                for a, b, t in lrel_sb:
                    if a <= c < b:
                        assert c + nb <= b
                        return t[:, :, c - a:c - a + nb]
                raise AssertionError(c)
            iota_t = constp.tile([128, OHB, 128], bf16)
            nc.gpsimd.iota(iota_t[:], pattern=[[0, OHB], [1, 128]], base=0,
                           channel_multiplier=0,
                           allow_small_or_imprecise_dtypes=True)

            gt = None
            oha = None
            ohb = None
            st = None
            c = 0       # global pair-chunk counter
            for t in range(TPC):
                ps = pspool.tile([128, D], f32, space="PSUM")
                K = int(S[t])
                for j in range(K):
                    if c % GCH == 0:
                        ci = c // GCH
                        gt = gpool.tile([128, GCH, 2 * D], bf16)
                        nc.gpsimd.dma_gather(
                            gt[:], tdall[:], rows_slice(ci),
                            GIDX, GIDX, 2 * D, queue_num=ci % 4)
                    if c % OHB == 0:
                        nb = min(OHB, nch - c)
                        oha = ohpool.tile([128, 2, OHB, 128], f8)
                        nc.vector.tensor_tensor(
                            out=oha[:, :, :nb, :],
                            in0=lrel_slice(c, nb)[:, :, :, None].to_broadcast(
                                [128, 2, nb, 128]),
                            in1=iota_t[:, None, :nb, :].to_broadcast(
                                [128, 2, nb, 128]),
                            op=mybir.AluOpType.is_equal)
                    nc.tensor.matmul(out=ps[:], lhsT=oha[:, 0, c % OHB, :],
                                     rhs=gt[:, c % GCH, 0:D],
                                     start=(j == 0), stop=False)
                    nc.tensor.matmul(out=ps[:], lhsT=oha[:, 1, c % OHB, :],
                                     rhs=gt[:, c % GCH, D:2 * D],
                                     start=False, stop=(j == K - 1))
                    c += 1
                if t % OB == 0:
                    st = stpool.tile([128, OB, D], bf16)
                nc.any.tensor_copy(st[:, t % OB, :], ps[:])
                if t % OB == OB - 1:
                    t0 = t - (OB - 1)
                    nc.sync.dma_start(out[:, t0:t0 + OB, :], st[:])
            assert c == nch
    nc.compile()
    return nc


def _pair_tile(r, l):
    """Fixed-grid pairing of one tile's row-sorted events.

    Returns (idx, cellA, cellB) int32 arrays, one entry per slot."""
    n = len(r)
    idxs, ca, cb = [], [], []
    k = 0
    half = n // 2
    if half:
        re, ro = r[0:2 * half:2].astype(np.int64), r[1:2 * half:2].astype(np.int64)
        le, lo_ = l[0:2 * half:2], l[1:2 * half:2]
        gap = ro - re
        ok = gap < NDELTA
        # paired slots
        idxs.append((gap[ok] * TROWS + re[ok]))
        ca.append(le[ok])
        cb.append(lo_[ok])
        # broken pairs -> two singles each
        for rr, ll in ((re[~ok], le[~ok]), (ro[~ok], lo_[~ok])):
            idxs.append(rr)
            ca.append(ll)
            cb.append(np.full(len(rr), -1, np.int64))
    if n % 2:
        idxs.append(np.array([int(r[-1])], np.int64))
        ca.append(np.array([int(l[-1])], np.int64))
        cb.append(np.array([-1], np.int64))
    if not idxs:
        return (np.zeros(0, np.int64),) * 3
    idx = np.concatenate(idxs)
    cA = np.concatenate(ca)
    cB = np.concatenate(cb)
    o = np.argsort(idx, kind="stable")   # ascending HBM addresses
    return idx[o], cA[o], cB[o]


def _marshal(event_cell, event_row):
    ecell = np.asarray(event_cell).astype(np.int64)
    erow = np.asarray(event_row).astype(np.int64)
    order = np.argsort(ecell, kind="stable")
    scell = ecell[order]
    srow = erow[order].astype(np.int64)

    ntiles = NCELLS // 128
    bounds = np.searchsorted(scell, np.arange(ntiles + 1) * 128)
    counts = np.diff(bounds)

    # per-tile slot lists (events row-sorted, fixed-grid paired)
    tile_slots = []
    for t in range(ntiles):
        s, n = int(bounds[t]), int(counts[t])
        rr, ll = srow[s:s + n], scell[s:s + n] & 127
        ro = np.argsort(rr, kind="stable")
        tile_slots.append(_pair_tile(rr[ro], ll[ro]))
    k2 = np.array([max(1, -(-len(ts[0]) // 128)) for ts in tile_slots])

    # snake-deal tiles (sorted by chunk count desc) to cores
    deal = np.argsort(-k2, kind="stable")
    assign = [[] for _ in range(NCORES)]
    for rank, t in enumerate(deal):
        r = rank % (2 * NCORES)
        cidx = r if r < NCORES else 2 * NCORES - 1 - r
        assign[cidx].append(int(t))
    pos_tiles = [sorted(ts, key=lambda t: (-k2[t], t)) for ts in assign]
    perm = []
    for a, b in zip(range(TPC // 2), reversed(range(TPC // 2, TPC))):
        perm += [a, b]
    pos_tiles = [[ts[i] for i in perm] for ts in pos_tiles]
    S = np.max(np.stack([[k2[t] for t in ts] for ts in pos_tiles]), axis=0)
    S = S.astype(np.int64)
    S[-1] += (-int(S.sum())) % GCH
    nch = int(S.sum())
    off = np.concatenate([[0], np.cumsum(S)])

    in_maps = []
    for cidx in range(NCORES):
        slots_p = np.zeros(nch * 128, np.int16)     # idx 0 padding (row 0)
        lrel_p = np.full((2, nch * 128), -1.0, np.float32)
        for p, t in enumerate(pos_tiles[cidx]):
            idx, cA, cB = tile_slots[t]
            n = len(idx)
            base = int(off[p]) * 128
            slots_p[base:base + n] = idx.astype(np.int16)
            lrel_p[0, base:base + n] = cA
            lrel_p[1, base:base + n] = cB
        wr = slots_p.reshape(-1, GIDX).reshape(-1, GIDX // 16, 16)
        wr = wr.transpose(0, 2, 1).reshape(-1, 16, GIDX // 16)
        wr = np.concatenate(list(wr), axis=1)
        wr = np.tile(wr, (8, 1))
        lc = lrel_p.reshape(2, nch, 128).transpose(2, 0, 1)  # [128, 2, nch]
        in_maps.append({
            "rows_w": np.ascontiguousarray(wr),
            "lrel": np.ascontiguousarray(lc.astype(ml_dtypes.bfloat16)),
        })
    return in_maps, tuple(int(x) for x in S), pos_tiles


def kernel(table, event_cell, event_row, _want_trace=False):
    from concourse.bass_utils import run_bass_kernel_spmd

    tabbf = np.asarray(table, dtype=np.float32).astype(ml_dtypes.bfloat16)
    td = np.empty((NDELTA, TROWS, 2 * D), dtype=ml_dtypes.bfloat16)
    ar = np.arange(TROWS)
    for dlt in range(NDELTA):
        td[dlt, :, :D] = tabbf
        td[dlt, :, D:] = tabbf[np.minimum(ar + dlt, TROWS - 1)]
    td = np.ascontiguousarray(td.reshape(NDELTA * TROWS, 2 * D))

    in_maps, S, pos_tiles = _marshal(event_cell, event_row)
    for m in in_maps:
        m["tdall"] = td

    if S not in _compiled:
        _compiled[S] = _build(S)
    nc = _compiled[S]

    kw = {"trace": True} if _want_trace else {}
    res = run_bass_kernel_spmd(nc, in_maps, core_ids=list(range(NCORES)), **kw)
    full = np.empty((NCELLS // 128, 128, D), np.float32)
    for cidx in range(NCORES):
        co = np.asarray(res.results[cidx]["out"]).astype(np.float32)
        full[np.array(pos_tiles[cidx])] = co.transpose(1, 0, 2)
    out = full.reshape(B, W, H, L, D)
    if _want_trace:
        return out, res
    return out
